# revision 16
# baseline (speedup 1.0000x reference)
"""Trainium2 Bass kernel for CircleProjectionLayer (ball projection, r=1).

out = center + d * min(1, 1/||d||),  d = x - center,  shapes [8388608, 3] f32.

The harness tolerance is rel_err < 2e-2 against an output scale of ~4.6; an
end-to-end fp16 pipeline measures ~8e-4 relative error — so the whole kernel
runs in fp16.  That halves HBM traffic (the DMA roofline) and doubles DVE
throughput (fp16 tensor_tensor runs in 2x perf mode).

Sharding: pure data parallel — batch split 8 ways, one shard per NeuronCore.
Host casts f32 -> fp16 before upload and fp16 -> f32 after download.

Per-core layout: the [1048576, 3] fp16 shard viewed flat as [128, 24576];
chunks of W fp16 elements per partition stream through SBUF.

Engine split (all fp16, per chunk of R = W/3 rows), chosen from HW
microbenchmarks (per-op costs include the unavoidable DVE pipeline drain):
  DVE   : d = x-c (dense 2x); ss clamp (tensor_scalar max vs eps, 4x);
          m = d * scale via ONE broadcast-input mul (stride-0 inner dim —
          measured 4.1us/chunk vs 7.7us for three strided muls);
          a head split of out = m + c (dense 2x)
  ACT   : a split of sq = Square(d) (dense interleaved); the scale chain
          Ln -> Relu -> Exp(-0.5*.) == min(1, rsqrt(ss)) (exact clamp at 1),
          BATCHED over B_C chunks so the unavoidable exp<->ln activation
          table loads (~2.7us each; Ln and Exp never share a table set)
          amortize; out-DMA triggers (HWDGE)
  GPSIMD: row sums of sq (two strided adds; Pool's 2-input penalty is
          smallest on strided work) and the tail of out = m + c
  DMA   : x-in + center-in on the SP HWDGE ring; out on the ACT HWDGE ring
          (SWDGE out-DMA would cost the Pool sequencer ~4us/chunk of
          descriptor generation).

Emission is software-pipelined via an explicit tick plan (modulo schedule):
stage s of chunk i is emitted at tick i+s so every engine's FIFO queue
interleaves stages of different chunks — chunk-grouped emission lockstepped
the engines (each queue head waiting on the same chunk's cross-engine dep)
and measured ~2x slower.  The For_i loop used for benchmarking drains all
engines at its back-edge, so the graded metric is single-shot latency:
chunk count trades steady-state op overhead against pipeline fill/drain.
"""

import sys

sys.path.insert(0, "/opt/trn_rl_repo")

from contextlib import ExitStack

import numpy as np

import concourse.bass as bass
import concourse.tile as tile
from concourse import bacc, mybir
from concourse.bass_utils import run_bass_kernel_spmd
from concourse.hw_specs import get_activation_tables

F16 = mybir.dt.float16
AF = mybir.ActivationFunctionType
ALU = mybir.AluOpType

B = 8388608
N_CORES = 8
B_CORE = B // N_CORES          # 1048576 rows per core
P = 128
FPP = B_CORE * 3 // P          # 24576 fp16 elements per partition

IN_DTYPE = np.float16

_EPS = 6.1e-5                  # smallest normal fp16; keeps Ln's input sane
_ACT_SET = "natural_log_exp_and_others"


def _preload_act_table(nc):
    tables = list(get_activation_tables(nc.m.arch).keys())
    set_id = tables.index(_ACT_SET)
    inst = mybir.InstLoadActFuncSet(
        name=nc.get_next_instruction_name(), act_func_set_id=set_id, ins=[], outs=[]
    )
    return nc.scalar.add_instruction(inst)


def _bcast3(src_2d, like_3d):
    """[P, r] -> [P, r, 3] AP with stride-0 inner dim (free broadcast)."""
    v = src_2d.rearrange("p (r one) -> p r one", one=1)
    bi, _ = bass.broadcast_tensor_aps(v, like_3d)
    return bi


def _build(W=3072, schedule=None, loop_reps=1, py_reps=1,
           alpha=0.375, f_dve=0.25, b_c=4, preload_act=True, out_ring="act",
           bufs=None):
    """alpha: column fraction of the Square on ACT (rest DVE).
    f_dve: column fraction of the final add on DVE (rest Pool).
    b_c: chunks per batched Ln/Relu/Exp chain (amortizes table loads)."""
    if schedule is None:
        assert W % 6 == 0 and FPP % W == 0
        schedule = [W] * (FPP // W)
    assert sum(schedule) == FPP and all(w == schedule[0] for w in schedule), (
        "v6 emitter assumes uniform chunks")
    W = schedule[0]
    n = len(schedule) * py_reps
    assert n % b_c == 0
    R = W // 3

    nc = bacc.Bacc("TRN2", target_bir_lowering=False, debug=False)

    x = nc.dram_tensor("x", [B_CORE, 3], F16, kind="ExternalInput")
    c = nc.dram_tensor("center", [B_CORE, 3], F16, kind="ExternalInput")
    o = nc.dram_tensor("out", [B_CORE, 3], F16, kind="ExternalOutput")

    xr = x.ap().rearrange("(p f) c -> p (f c)", p=P)
    cr = c.ap().rearrange("(p f) c -> p (f c)", p=P)
    orr = o.ap().rearrange("(p f) c -> p (f c)", p=P)

    if bufs is None:
        bufs = dict(x=3, c=9, d=8, sq=3, m=3, ss=2, sc=2)

    with tile.TileContext(nc) as tc, ExitStack() as ctx:
        if preload_act:
            _preload_act_table(nc)

        xp = ctx.enter_context(tc.tile_pool(name="xp", bufs=bufs["x"]))
        cp = ctx.enter_context(tc.tile_pool(name="cp", bufs=bufs["c"]))
        dp = ctx.enter_context(tc.tile_pool(name="dp", bufs=bufs["d"]))
        sqp = ctx.enter_context(tc.tile_pool(name="sqp", bufs=bufs["sq"]))
        mp = ctx.enter_context(tc.tile_pool(name="mp", bufs=bufs["m"]))
        ssp = ctx.enter_context(tc.tile_pool(name="ssp", bufs=bufs["ss"]))
        scp = ctx.enter_context(tc.tile_pool(name="scp", bufs=bufs["sc"]))

        import contextlib
        loop_cm = tc.For_i(0, loop_reps, 1) if loop_reps > 1 else contextlib.nullcontext()
        with loop_cm:
            _emit_v6(nc, n, W, R, alpha, f_dve, b_c,
                     xp, cp, dp, sqp, mp, ssp, scp, xr, cr, orr, out_ring)

    nc.compile()
    return nc


def _emit_v6(nc, n, W, R, alpha, f_dve, b_c,
             xp, cp, dp, sqp, mp, ssp, scp, xr, cr, orr, out_ring):
    FPW = FPP // W                      # chunks per pass over the tensor
    st = [{} for _ in range(n)]
    bt = [{} for _ in range(n // b_c)]  # per-batch chain tiles
    o_dma = {"sp": nc.sync, "act": nc.scalar, "pool": nc.gpsimd}[out_ring]

    wa = int(W * alpha) // 6 * 6        # ACT share of the Square
    wf = int(W * f_dve) // 6 * 6        # DVE share of the final add

    def dma_in(i):
        off = (i % FPW) * W
        xt = xp.tile([P, W], F16, name="xt", tag="xt")
        nc.sync.dma_start(xt[:, :], xr[:, off : off + W])
        ct = cp.tile([P, W], F16, name="ct", tag="ct")
        nc.sync.dma_start(ct[:, :], cr[:, off : off + W])
        st[i].update(xt=xt, ct=ct)

    def sub(i):
        dt = dp.tile([P, W], F16, name="dt", tag="dt")
        nc.vector.tensor_sub(dt[:, :], st[i]["xt"][:, :], st[i]["ct"][:, :])
        st[i]["dt"] = dt

    def squares(i):
        dt = st[i]["dt"]
        sq = sqp.tile([P, W], F16, name="sq", tag="sq")
        if wa > 0:
            nc.scalar.activation(sq[:, :wa], dt[:, :wa], AF.Square)
        if wa < W:
            nc.vector.tensor_mul(sq[:, wa:], dt[:, wa:], dt[:, wa:])
        st[i]["sq"] = sq

    def sums(i):
        b, j = divmod(i, b_c)
        sq3 = st[i]["sq"].rearrange("p (r c) -> p r c", c=3)
        if j == 0:
            bt[b]["ss"] = ssp.tile([P, b_c * R], F16, name="ss", tag="ss")
            bt[b]["sc"] = scp.tile([P, b_c * R], F16, name="sc", tag="sc")
        ta = mp.tile([P, R], F16, name="ta", tag="ta")
        nc.gpsimd.tensor_add(ta[:, :], sq3[:, :, 0], sq3[:, :, 1])
        tb = mp.tile([P, R], F16, name="tb", tag="tb")
        nc.gpsimd.tensor_add(tb[:, :], ta[:, :], sq3[:, :, 2])
        # clamp on DVE (tensor_scalar runs 4x) into this chunk's batch slice
        nc.vector.tensor_scalar_max(
            bt[b]["ss"][:, j * R : (j + 1) * R], tb[:, :], _EPS)

    def chain(b):
        ss, sc = bt[b]["ss"], bt[b]["sc"]
        # scale = exp(-0.5*relu(ln(ss))) == min(1, rsqrt(ss)), exact clamp.
        nc.scalar.activation(sc[:, :], ss[:, :], AF.Ln)
        nc.scalar.activation(ss[:, :], sc[:, :], AF.Relu)
        nc.scalar.activation(sc[:, :], ss[:, :], AF.Exp, scale=-0.5)

    def emul(i):
        b, j = divmod(i, b_c)
        dt = st[i]["dt"]
        d3 = dt.rearrange("p (r c) -> p r c", c=3)
        mt = mp.tile([P, W], F16, name="mt", tag="mt")
        m3 = mt.rearrange("p (r c) -> p r c", c=3)
        s_b = _bcast3(bt[b]["sc"][:, j * R : (j + 1) * R], d3)
        nc.vector.tensor_mul(m3, d3, s_b)
        st[i]["mt"] = mt

    def fadd(i):
        dt, mt, ct = st[i]["dt"], st[i]["mt"], st[i]["ct"]
        if wf > 0:
            nc.vector.tensor_add(dt[:, :wf], mt[:, :wf], ct[:, :wf])
        if wf < W:
            nc.gpsimd.tensor_add(dt[:, wf:], mt[:, wf:], ct[:, wf:])

    def dma_out(i):
        off = (i % FPW) * W
        o_dma.dma_start(orr[:, off : off + W], st[i]["dt"][:, :])
        st[i].clear()

    # Build the tick plan (producer-before-consumer within each tick).
    n_ticks = n + b_c + 10
    plan = [[] for _ in range(n_ticks)]
    for i in range(n):
        b = i // b_c
        t_chain = b_c * b + (b_c - 1) + 4    # one tick after last sums of b
        plan[i + 0].append((7, dma_in, i))
        plan[i + 1].append((6, sub, i))
        plan[i + 2].append((5, squares, i))
        plan[i + 3].append((4, sums, i))
        if i % b_c == b_c - 1:
            plan[t_chain].append((3, chain, b))
        plan[t_chain + 0].append((2, emul, i))
        plan[t_chain + 1].append((1, fadd, i))
        plan[t_chain + 2].append((0, dma_out, i))
    for t in range(n_ticks):
        for _, fn, arg in sorted(plan[t], key=lambda e: -e[0]):
            fn(arg)


_NC = None

_SCHEDULE = [3072] * 8


def _get_nc():
    global _NC
    if _NC is None:
        _NC = _build(schedule=_SCHEDULE)
    return _NC


def kernel(**inputs):
    x = np.asarray(inputs["x"], dtype=np.float32)
    center = np.asarray(inputs["center"], dtype=np.float32)
    assert x.shape == (B, 3) and center.shape == (B, 3)

    x16 = x.astype(np.float16)
    c16 = center.astype(np.float16)
    xs = x16.reshape(N_CORES, B_CORE, 3)
    cs = c16.reshape(N_CORES, B_CORE, 3)
    in_maps = [
        {"x": np.ascontiguousarray(xs[i]), "center": np.ascontiguousarray(cs[i])}
        for i in range(N_CORES)
    ]

    nc = _get_nc()
    res = run_bass_kernel_spmd(nc, in_maps, list(range(N_CORES)))
    out = np.concatenate([res.results[i]["out"] for i in range(N_CORES)], axis=0)
    return out.astype(np.float32)


if __name__ == "__main__":
    nc = _get_nc()
    print("build ok")
